# revision 23
# baseline (speedup 1.0000x reference)
"""BinaryConv2D Trainium2 kernel.

Reference op: out = conv2d(sign(clip(x,-1,1)), sign(clip(w,-1,1))),
NHWC x HWIO -> NHWC, SAME padding, stride 1, fp32.

sign() of a nonzero float is exactly +-1, exactly representable in
bf16/fp8e4, and every partial sum is an integer bounded by 3*3*256 =
2304 (< 2^24), so the conv is computed EXACTLY with fp8 DoubleRow
matmuls (2 cin-chunks contracted per pass) accumulating into fp32 PSUM.

Sharding: data-parallel over batch. 32 images / 8 cores = 4 images per
core; full weights replicated. No collectives.

Per-core pipeline:
  1. One SWDGE cast-DMA per image: raw fp32 NHWC -> bf16 into a
     zero-padded DRAM staging grid (58x58 padded rows; SAME padding =
     zero borders; sign survives the cast). All casts issue upfront.
  2. One DMA-transpose per (image, cin-chunk): staged [3392 pix, 128
     cin] bf16 -> SBUF [128 cin, 3392 pix] (channel-major). HWDGE
     queues carry ONLY transposes (other DMAs ride SWDGE) to avoid
     xbar-mode serialization.
  3. Binarize post-transpose on ACT (sign -> fp8), weights on DVE.
  4. Conv as implicit GEMM, fp8 DoubleRow: psum[cout=128, 464]
     accumulates 9 taps (contraction 256 per matmul); rhs is a
     contiguous 464-wide window of the padded pixel stream (the 2 pad
     columns per row accumulate junk, dropped at evacuation).
  5. PSUM -> SBUF (strided DVE copy keeps 56 of 58 cols) -> DRAM out
     [2, 128, 12544] cout-major; host transposes back to NHWC while
     unsharding.
"""

import numpy as np

import concourse.bass as bass
import concourse.mybir as mybir
from concourse import bacc
from concourse.tile import TileContext
from concourse.bass_utils import run_bass_kernel_spmd

F32 = mybir.dt.float32
BF16 = mybir.dt.bfloat16
FP8 = mybir.dt.float8e4

N_CORES = 8
N_IMG = 4            # images per core
H = W = 56
CIN = COUT = 256
NPIX = H * W                      # 3136 pixels per image
PW = W + 2                        # 58: padded row width
PIXPAD = PW * (H + 2)             # 3364 padded pixels
PIXPAD_AL = 3392                  # aligned up to 16 for DMA transpose
CH = 3456                         # act chunk stride (room for AP construction)
ROWBLK = 8                        # output rows per psum tile
NBLK = H // ROWBLK                # 7
NTP = ROWBLK * W                  # 448 output pixels per psum tile


def build(nc: bass.Bass, mode: str = "fp8"):
    x_d = nc.dram_tensor("x", [N_IMG * NPIX, CIN], F32, kind="ExternalInput")
    w_d = nc.dram_tensor("w", [9 * CIN, COUT], F32, kind="ExternalInput")
    y_d = nc.dram_tensor("y", [2, 128, N_IMG * NPIX], F32, kind="ExternalOutput")

    NT = 464 if mode == "fp8" else NTP             # psum free size

    with TileContext(nc) as tc:
        with (
            tc.tile_pool(name="wpool", bufs=1) as wpool,
            tc.tile_pool(name="wstage", bufs=1) as wstage,
            tc.tile_pool(name="zpool", bufs=1) as zpool,
            tc.tile_pool(name="stage", bufs=4, space="DRAM") as dpool,
            tc.tile_pool(name="xb", bufs=3) as xbpool,
            tc.tile_pool(name="act", bufs=2) as actpool,
            tc.tile_pool(name="psum", bufs=8, space="PSUM") as psumpool,
            tc.tile_pool(name="out", bufs=6) as outpool,
        ):
            # ---- weights: one DMA + binarize on DVE (keeps the ACT queue
            # free for transposes/signs). sign = ((w>=0)*2) - 1.
            # layout [p, g=(t,i), c]: partition p holds w row g*128+p.
            wst = wstage.tile([128, 18, COUT], F32)
            nc.sync.dma_start(
                out=wst[:], in_=w_d[:].rearrange("(g p) c -> p g c", p=128)
            )
            wge = wstage.tile([128, 18, COUT], F32)
            nc.vector.tensor_scalar(
                wge[:], wst[:], 0.0, 2.0,
                mybir.AluOpType.is_ge, mybir.AluOpType.mult,
            )
            if mode == "fp8":
                # DoubleRow block pairing: partition p holds cin (i*128+p)
                wb8 = wpool.tile([128, 9, 2, COUT], FP8)
                nc.vector.tensor_scalar_add(
                    wb8[:].rearrange("p t i c -> p (t i) c"), wge[:], -1.0
                )
            else:
                wb = wpool.tile([128, 18 * COUT], BF16)
                nc.vector.tensor_scalar_add(
                    wb[:].rearrange("p (g c) -> p g c", c=COUT), wge[:], -1.0
                )

            zt = zpool.tile([57, 512], BF16)
            nc.gpsimd.memset(zt[:], 0.0)

            stages = [
                dpool.tile([PIXPAD_AL, CIN], BF16, tag="stage", name=f"stage{n}")
                for n in range(N_IMG)
            ]

            # ---- zero borders (also SWDGE; HWDGE stays transpose-only)
            for n in range(N_IMG):
                sflat = stages[n][:].rearrange("r c -> (r c)")
                nc.gpsimd.dma_start(
                    out=sflat[0 : PW * CIN].rearrange("(a b) -> a b", b=512),
                    in_=zt[0:29, :],
                )
                nc.gpsimd.dma_start(
                    out=sflat[57 * PW * CIN : 58 * PW * CIN].rearrange(
                        "(a b) -> a b", b=512
                    ),
                    in_=zt[0:29, :],
                )
                # right-pad of row r + left-pad of row r+1, r=0..56
                nc.gpsimd.dma_start(
                    out=sflat[57 * CIN : 57 * CIN + 57 * PW * CIN]
                    .rearrange("(r x) -> r x", x=PW * CIN)[:, 0:512],
                    in_=zt[:, :],
                )
                # alignment tail rows (read by the transpose, not matmuls)
                nc.gpsimd.dma_start(
                    out=sflat[PIXPAD * CIN : PIXPAD_AL * CIN].rearrange(
                        "(a b) -> a b", b=512
                    ),
                    in_=zt[0:14, :],
                )

            # ---- all casts upfront on SWDGE: raw fp32 -> bf16 padded rows
            for n in range(N_IMG):
                sflat = stages[n][:].rearrange("r c -> (r c)")
                off = (PW + 1) * CIN
                dst = sflat[off : off + H * PW * CIN].rearrange(
                    "(r x) -> r x", x=PW * CIN
                )[:, 0 : W * CIN]
                nc.gpsimd.dma_start(
                    out=dst,
                    in_=x_d[n * NPIX : (n + 1) * NPIX, :].rearrange(
                        "(r w) c -> r (w c)", w=W
                    ),
                )

            def prep(n):
                """Transpose image n to channel-major [128 cin, pix] and
                binarize (ACT sign, casting to the matmul dtype)."""
                if mode == "fp8":
                    act8 = actpool.tile([128, 2, CH], FP8, tag="act8")
                else:
                    act8 = actpool.tile([128, 2 * CH], BF16, tag="act8")
                for ki in range(2):
                    actb = xbpool.tile([128, PIXPAD_AL], BF16, tag="actb", bufs=3)
                    nc.scalar.dma_start(
                        out=actb[:],
                        in_=stages[n][:, ki * 128 : (ki + 1) * 128],
                        transpose=True,
                    )
                    if mode == "fp8":
                        nc.scalar.sign(act8[:, ki, 0:PIXPAD_AL], actb[:])
                    else:
                        nc.scalar.sign(
                            act8[:, ki * CH : ki * CH + PIXPAD_AL], actb[:]
                        )
                return act8

            acts = {0: prep(0)}
            for n in range(N_IMG):
                if n + 1 < N_IMG:
                    acts[n + 1] = prep(n + 1)
                a = acts[n]
                for m in range(2):          # cout chunk
                    for j in range(NBLK):   # 8-row output block
                        psum = psumpool.tile([128, NT], F32)
                        if mode == "fp8":
                            for t in range(9):
                                dy, dx = t // 3 - 1, t % 3 - 1
                                base = (ROWBLK * j + 1 + dy) * PW + 1 + dx
                                nc.tensor.matmul(
                                    psum[:],
                                    wb8[:, t, :, m * 128 : (m + 1) * 128],
                                    a[:, :, base : base + NT],
                                    start=(t == 0),
                                    stop=(t == 8),
                                    perf_mode=mybir.MatmulPerfMode.DoubleRow,
                                )
                        else:
                            first = True
                            for ki in range(2):
                                for t in range(9):
                                    dy, dx = t // 3 - 1, t % 3 - 1
                                    base = (
                                        ki * CH + (ROWBLK * j + 1 + dy) * PW + 1 + dx
                                    )
                                    rhs = a[:, base : base + ROWBLK * PW].rearrange(
                                        "p (r c) -> p r c", c=PW
                                    )[:, :, 0:W]
                                    idx = t * 2 + ki
                                    nc.tensor.matmul(
                                        psum[:],
                                        wb[:, idx * COUT + m * 128 : idx * COUT + (m + 1) * 128],
                                        rhs,
                                        start=first,
                                        stop=(ki == 1 and t == 8),
                                    )
                                    first = False
                        ot = outpool.tile([128, NTP], F32)
                        if mode == "fp8":
                            nc.vector.tensor_copy(
                                ot[:].rearrange("p (r c) -> p r c", c=W),
                                psum[:].rearrange("p (r c) -> p r c", c=PW)[:, :, 0:W],
                            )
                        else:
                            nc.vector.tensor_copy(ot[:], psum[:])
                        nc.gpsimd.dma_start(
                            out=y_d[m][:, n * NPIX + j * NTP : n * NPIX + (j + 1) * NTP],
                            in_=ot[:],
                        )
    return nc


def _run(x: np.ndarray, w: np.ndarray, trace: bool = False, mode: str = "fp8"):
    """x: (32,56,56,256) f32, w: (3,3,256,256) f32 -> (out, BassKernelResults)."""
    nc = bacc.Bacc(None, target_bir_lowering=False, debug=False)
    build(nc, mode=mode)
    nc.finalize()  # Bacc.compile: legalizes multi-wait insts into event sems
    wf = np.ascontiguousarray(w.reshape(9 * CIN, COUT))
    in_maps = []
    for c in range(N_CORES):
        xs = np.ascontiguousarray(
            x[c * N_IMG : (c + 1) * N_IMG].reshape(N_IMG * NPIX, CIN)
        )
        in_maps.append({"x": xs, "w": wf})
    res = run_bass_kernel_spmd(nc, in_maps, core_ids=list(range(N_CORES)), trace=trace)
    outs = []
    for c in range(N_CORES):
        y = res.results[c]["y"]  # [2, 128, 12544]
        o = (
            y.reshape(2, 128, N_IMG, H, W)
            .transpose(2, 3, 4, 0, 1)
            .reshape(N_IMG, H, W, COUT)
        )
        outs.append(o)
    return np.concatenate(outs, axis=0).astype(np.float32), res


def kernel(**inputs) -> np.ndarray:
    x = np.asarray(inputs["inputs"], dtype=np.float32)
    w = np.asarray(inputs["kernel"], dtype=np.float32)
    out, _ = _run(x, w, trace=False, mode="fp8")
    return out
